# revision 11
# baseline (speedup 1.0000x reference)
"""JointAtt (dense_cnn) Trainium2 Bass kernel — bf16 pipelined version.

Reference computation (per batch n, group g of 4, cg=128 channels, 64x64):
    gh = mean_w x          # (cg, h)
    gw = mean_h x          # (cg, w)
    y  = BN(W1 @ concat(gh, gw) + b1)        # (16, h+w)
    y  = hswish(y) = y * relu6(y+3)/6
    a_h = sigmoid(Wh @ y[:, :h] + bh)        # (cg, h)
    a_w = sigmoid(Ww @ y[:, h:] + bw)        # (cg, w)
    out = x * a_h[:, :, None] * a_w[:, None, :]
    followed by channel shuffle: c' = (c % 4) * 128 + c // 4

Kernel strategy (8 NeuronCores, data-parallel over batch: 2 batches/core):
  - x and out travel as bf16 (host converts): halves HBM traffic and
    enables the DVE 2x_1p mode on the big elementwise multiplies.
  - Per (n, g) slice: x loaded as SBUF [128, 4096] bf16, channel order
    permuted so the final store is the channel shuffle applied contiguously
    (weights permuted on the host to match).
  - Pooling sums fused with the conv1 contraction on the TensorEngine
    (PSUM accumulation, bf16 full rate). Yh accumulates w-octaves with
    n=(h,8w) reads; Yw accumulates h-octaves with fully contiguous
    n=(8h,w) slab reads (h-residues reduced later on DVE).
  - BN scale/bias, the 1/64 pooling mean and the 1/6 hswish divisor are
    folded into the weights on the host.
  - hswish entirely on DVE: T = max(Y+b+3, 0); HS = (T-3)*min(T,6).
  - a_h sigmoid materialized as a broadcast [128, 64, 64] straight out of
    PSUM on the Activation engine (one op); a_w kept [128, 64] and fed to
    DVE as a broadcast access pattern (inner dim stays packed -> 2x mode).
  - Software pipeline with per-stage step offsets so no engine queue
    head-blocks: load(k) | pool-mm(k-1) | reduce+hswish(k-2) |
    att-mm(k-3) | sigmoids(k-4) | big-mults(k-5) | store(k-5).
"""

import numpy as np
import ml_dtypes

import concourse.bass as bass
import concourse.bacc as bacc
import concourse.mybir as mybir
import concourse.tile as tile
from concourse.bass_utils import run_bass_kernel_spmd

F32 = mybir.dt.float32
BF16 = mybir.dt.bfloat16

N_CORES = 8
NB = 2          # batches per core
C = 512
G = 4           # groups
CG = 128        # channels per group
H = 64
W = 64
HW = H * W
MIP = 16        # conv1 output channels
EPS = 1e-5
NSTEP = NB * G  # pipeline iterations per core

# Partition p holds input channel cc = PERM[p] (within its group).
# p = 32*r + q  <->  cc = 4*q + r, so that output channels are contiguous.
PERM = np.array([4 * (p % 32) + p // 32 for p in range(CG)], dtype=np.int64)

_NC_CACHE = None


def _build_bass():
    nc = bacc.Bacc(None, target_bir_lowering=False)

    x_d = nc.dram_tensor("x", [NB, C, H, W], BF16, kind="ExternalInput")
    w1t_d = nc.dram_tensor("w1t", [CG, MIP], BF16, kind="ExternalInput")
    wht_d = nc.dram_tensor("wht", [MIP, CG], BF16, kind="ExternalInput")
    wwt_d = nc.dram_tensor("wwt", [MIP, CG], BF16, kind="ExternalInput")
    bact_d = nc.dram_tensor("bact", [MIP, 1], F32, kind="ExternalInput")
    bh_d = nc.dram_tensor("bh", [CG, 1], F32, kind="ExternalInput")
    bw_d = nc.dram_tensor("bw", [CG, 1], F32, kind="ExternalInput")
    out_d = nc.dram_tensor("out", [NB, C, H, W], BF16, kind="ExternalOutput")

    Relu = mybir.ActivationFunctionType.Relu
    Sigmoid = mybir.ActivationFunctionType.Sigmoid
    AX = mybir.AxisListType.X
    ADD = mybir.AluOpType.add
    MAX = mybir.AluOpType.max
    MIN = mybir.AluOpType.min
    MULT = mybir.AluOpType.mult

    x_f = x_d[:].rearrange("b c h w -> b c (h w)")
    o_f = out_d[:].rearrange("b c h w -> b c (h w)")

    with tile.TileContext(nc) as tc:
        with (
            tc.tile_pool(name="consts", bufs=1) as consts,
            tc.tile_pool(name="xp", bufs=7) as xp,
            tc.tile_pool(name="op", bufs=3) as op,
            tc.tile_pool(name="ahp", bufs=3) as ahp,
            tc.tile_pool(name="ps", bufs=3, space="PSUM") as ps,
            tc.tile_pool(name="ps2", bufs=2, space="PSUM") as ps2,
            tc.tile_pool(name="sm", bufs=12) as sm,
        ):
            w1t = consts.tile([CG, MIP], BF16)
            nc.sync.dma_start(out=w1t, in_=w1t_d[:])
            wht = consts.tile([MIP, CG], BF16)
            nc.sync.dma_start(out=wht, in_=wht_d[:])
            wwt = consts.tile([MIP, CG], BF16)
            nc.sync.dma_start(out=wwt, in_=wwt_d[:])
            bact = consts.tile([MIP, 1], F32)
            nc.sync.dma_start(out=bact, in_=bact_d[:])
            bh = consts.tile([CG, 1], F32)
            nc.sync.dma_start(out=bh, in_=bh_d[:])
            bw = consts.tile([CG, 1], F32)
            nc.sync.dma_start(out=bw, in_=bw_d[:])

            # Pipeline state per in-flight iteration.
            S = [dict() for _ in range(NSTEP)]

            def stg_load(k):
                bi, g = divmod(k, G)
                # 4 DMAs, each with an affine DRAM stride (channels r, r+4,
                # ...) -> partition block [32r, 32r+32): the non-affine
                # 1-DMA nested pattern defeats the 16-engine descriptor
                # spray (measured ~74 GB/s vs ~340 expected).
                X = xp.tile([CG, HW], BF16, name="X")
                for r in range(4):
                    nc.sync.dma_start(
                        out=X[32 * r : 32 * (r + 1)],
                        in_=x_f[bi, CG * g + r : CG * (g + 1) : 4],
                    )
                S[k]["X"] = X

            def stg_pool_mm(k):
                # YHW[m, 0, h, j] accumulates w-octaves of h-pooling;
                # YHW[m, 1, w, j] accumulates h-octaves (contiguous slab
                # reads, strided PSUM writes), j = the octave residue. Both
                # halves end j-minor so ONE stride-1 reduce finishes them.
                X3 = S[k]["X"].rearrange("p (h w) -> p h w", h=H)
                YHW = ps.tile([MIP, 2, H, 8], F32, name="YHW")
                for j in range(8):
                    nc.tensor.matmul(
                        YHW[:, 0],
                        w1t,
                        X3[:, :, 8 * j : 8 * (j + 1)],
                        start=(j == 0),
                        stop=(j == 7),
                    )
                Yw_v = YHW[:, 1].rearrange("p w j -> p j w")
                for j in range(8):
                    nc.tensor.matmul(
                        Yw_v,
                        w1t,
                        X3[:, 8 * j : 8 * (j + 1), :],
                        start=(j == 0),
                        stop=(j == 7),
                    )
                S[k]["YHW"] = YHW

            def stg_hswish(k):
                # Y = [Yh | Yw] (16, 128); then hswish with T = relu(ybn + 3):
                # ybn * relu6(ybn+3) == (T - 3) * min(T, 6)   (/6 in weights)
                Y = sm.tile([MIP, H + W], F32, name="Y")
                nc.vector.tensor_reduce(
                    out=Y.rearrange("p (d l) -> p d l", d=2),
                    in_=S[k]["YHW"],
                    axis=AX,
                    op=ADD,
                )
                T = sm.tile([MIP, H + W], F32, name="T")
                nc.vector.tensor_scalar(
                    out=T, in0=Y, scalar1=bact[:], scalar2=0.0, op0=ADD, op1=MAX
                )
                T6 = sm.tile([MIP, H + W], F32, name="T6")
                nc.vector.tensor_scalar_min(T6, T, 6.0)
                HS = sm.tile([MIP, H + W], BF16, name="HS")
                nc.vector.scalar_tensor_tensor(
                    out=HS, in0=T, scalar=-3.0, in1=T6, op0=ADD, op1=MULT
                )
                S[k]["HS"] = HS

            def stg_att_mm(k):
                AHW_ps = ps2.tile([CG, H + W], F32, name="AHW_ps")
                nc.tensor.matmul(
                    AHW_ps[:, 0:H], wht, S[k]["HS"][:, 0:H], start=True, stop=True
                )
                nc.tensor.matmul(
                    AHW_ps[:, H:], wwt, S[k]["HS"][:, H:], start=True, stop=True
                )
                S[k]["AHW_ps"] = AHW_ps

            def stg_sigmoid(k):
                AHW_ps = S[k]["AHW_ps"]
                # a_w first: it unblocks the first big multiply after ~0.3us,
                # overlapping the 3.7us a_h materialization with TT1.
                AW = sm.tile([CG, W], BF16, name="AW")
                nc.scalar.activation(
                    out=AW, in_=AHW_ps[:, H:], func=Sigmoid, bias=bw[:]
                )
                # a_h sigmoid materialized as the broadcast [cg, h, w]; split
                # in halves so the second big multiply can chase it.
                AH = ahp.tile([CG, H, W], BF16, name="AH")
                for half in range(2):
                    h0 = half * (H // 2)
                    nc.scalar.activation(
                        out=AH[:, h0 : h0 + H // 2],
                        in_=AHW_ps[:, h0 : h0 + H // 2]
                        .unsqueeze(2)
                        .broadcast_to([CG, H // 2, W]),
                        func=Sigmoid,
                        bias=bh[:],
                    )
                S[k]["AH"], S[k]["AW"] = AH, AW

            def stg_mult(k):
                # out = x * a_w[., :, w] * a_h[., h, :]; both tensor_tensor
                # ops keep every operand's inner dim packed bf16 -> DVE 2x.
                X3 = S[k]["X"].rearrange("p (h w) -> p h w", h=H)
                OUT = op.tile([CG, HW], BF16, name="OUT")
                OUTr = OUT.rearrange("p (h w) -> p h w", h=H)
                aw_b = S[k]["AW"].unsqueeze(1).broadcast_to([CG, H, W])
                nc.vector.tensor_tensor(out=OUTr, in0=X3, in1=aw_b, op=MULT)
                AH = S[k]["AH"]
                for half in range(2):
                    h0 = half * (H // 2)
                    nc.vector.tensor_tensor(
                        out=OUTr[:, h0 : h0 + H // 2],
                        in0=OUTr[:, h0 : h0 + H // 2],
                        in1=AH[:, h0 : h0 + H // 2],
                        op=MULT,
                    )
                S[k]["OUT"] = OUT

            def stg_store(k):
                bi, g = divmod(k, G)
                OUT = S[k]["OUT"]
                # channel shuffle = 4 contiguous writes; triggers on the
                # gpsimd (Pool) SWDGE ring to keep HWDGE engines free.
                for r in range(4):
                    c0 = 128 * r + 32 * g
                    nc.gpsimd.dma_start(
                        out=o_f[bi, c0 : c0 + 32],
                        in_=OUT[32 * r : 32 * (r + 1)],
                    )

            # Software pipeline: stage s of iteration k runs in python step
            # k + OFF[s]. hswish leads the Vector queue each step so HS(k)
            # lands early; att-mm(k) (same step, PE) and the sigmoids (+1)
            # then never gate the next step's Vector work — every other
            # cross-engine edge is >= 1 full step old.
            stages = [
                (stg_load, 0),
                (stg_pool_mm, 1),
                (stg_hswish, 3),
                (stg_att_mm, 3),
                (stg_sigmoid, 4),
                (stg_mult, 5),
                (stg_store, 5),
            ]
            maxoff = max(off for _, off in stages)
            for step in range(NSTEP + maxoff):
                for fn, off in stages:
                    k = step - off
                    if 0 <= k < NSTEP:
                        fn(k)

    nc.finalize()
    return nc


def _get_nc():
    global _NC_CACHE
    if _NC_CACHE is None:
        _NC_CACHE = _build_bass()
    return _NC_CACHE


def _prep_weights(W1, b1, gamma, beta, mean, var, Wh, bh, Ww, bw):
    W1 = np.asarray(W1, np.float64)
    b1 = np.asarray(b1, np.float64)
    gamma = np.asarray(gamma, np.float64)
    beta = np.asarray(beta, np.float64)
    mean = np.asarray(mean, np.float64)
    var = np.asarray(var, np.float64)
    Wh = np.asarray(Wh, np.float64)
    Ww = np.asarray(Ww, np.float64)
    bh = np.asarray(bh, np.float64)
    bw = np.asarray(bw, np.float64)

    scale = gamma / np.sqrt(var + EPS)                    # (MIP,)
    w1eff = (W1 * scale[:, None]) / float(W)              # (MIP, CG); mean 1/64
    b1eff = scale * (b1 - mean) + beta                    # (MIP,)
    bact = (b1eff + 3.0).astype(np.float32)[:, None]      # (MIP, 1)

    BF = ml_dtypes.bfloat16
    w1t = np.ascontiguousarray(w1eff.T[PERM, :].astype(BF))            # (CG, MIP)
    wht = np.ascontiguousarray((Wh / 6.0)[PERM, :].T.astype(BF))       # (MIP, CG)
    wwt = np.ascontiguousarray((Ww / 6.0)[PERM, :].T.astype(BF))
    bh_p = np.ascontiguousarray(bh[PERM].astype(np.float32)[:, None])
    bw_p = np.ascontiguousarray(bw[PERM].astype(np.float32)[:, None])
    return w1t, wht, wwt, bact, bh_p, bw_p


def run(inputs: dict, trace: bool = False):
    """Run on 8 NeuronCores. Returns (out [16,512,64,64] fp32, results)."""
    x = np.asarray(inputs["x"], dtype=np.float32)
    n = x.shape[0]
    assert x.shape == (n, C, H, W) and n == N_CORES * NB, x.shape
    x_bf = np.ascontiguousarray(x.astype(ml_dtypes.bfloat16))

    w1t, wht, wwt, bact, bh_p, bw_p = _prep_weights(
        inputs["W1"], inputs["b1"], inputs["gamma"], inputs["beta"],
        inputs["mean"], inputs["var"], inputs["Wh"], inputs["bh"],
        inputs["Ww"], inputs["bw"],
    )

    nc = _get_nc()
    core_ids = list(range(N_CORES))
    in_maps = []
    for k in core_ids:
        in_maps.append(
            {
                "x": np.ascontiguousarray(x_bf[NB * k : NB * (k + 1)]),
                "w1t": w1t,
                "wht": wht,
                "wwt": wwt,
                "bact": bact,
                "bh": bh_p,
                "bw": bw_p,
            }
        )

    res = run_bass_kernel_spmd(nc, in_maps, core_ids, trace=trace)
    out = np.concatenate(
        [res.results[k]["out"].astype(np.float32) for k in core_ids], axis=0
    )
    return out, res


def kernel(**inputs) -> np.ndarray:
    out, _ = run(inputs, trace=False)
    return out


def exec_time_ns(res):
    return res.exec_time_ns


# revision 14
# speedup vs baseline: 1.0196x; 1.0196x over previous
"""JointAtt (dense_cnn) Trainium2 Bass kernel — bf16 pipelined version.

Reference computation (per batch n, group g of 4, cg=128 channels, 64x64):
    gh = mean_w x          # (cg, h)
    gw = mean_h x          # (cg, w)
    y  = BN(W1 @ concat(gh, gw) + b1)        # (16, h+w)
    y  = hswish(y) = y * relu6(y+3)/6
    a_h = sigmoid(Wh @ y[:, :h] + bh)        # (cg, h)
    a_w = sigmoid(Ww @ y[:, h:] + bw)        # (cg, w)
    out = x * a_h[:, :, None] * a_w[:, None, :]
    followed by channel shuffle: c' = (c % 4) * 128 + c // 4

Kernel strategy (8 NeuronCores, data-parallel over batch: 2 batches/core):
  - x and out travel as bf16 (host converts): halves HBM traffic and
    enables the DVE 2x_1p mode on the big elementwise multiplies.
  - Per (n, g) slice: x loaded as SBUF [128, 4096] bf16, channel order
    permuted so the final store is the channel shuffle applied contiguously
    (weights permuted on the host to match).
  - Pooling sums fused with the conv1 contraction on the TensorEngine
    (PSUM accumulation, bf16 full rate). Yh accumulates w-octaves with
    n=(h,8w) reads; Yw accumulates h-octaves with fully contiguous
    n=(8h,w) slab reads (h-residues reduced later on DVE).
  - BN scale/bias, the 1/64 pooling mean and the 1/6 hswish divisor are
    folded into the weights on the host.
  - hswish entirely on DVE: T = max(Y+b+3, 0); HS = (T-3)*min(T,6).
  - a_h sigmoid materialized as a broadcast [128, 64, 64] straight out of
    PSUM on the Activation engine (one op); a_w kept [128, 64] and fed to
    DVE as a broadcast access pattern (inner dim stays packed -> 2x mode).
  - Software pipeline with per-stage step offsets so no engine queue
    head-blocks: load(k) | pool-mm(k-1) | reduce+hswish(k-2) |
    att-mm(k-3) | sigmoids(k-4) | big-mults(k-5) | store(k-5).
"""

import numpy as np
import ml_dtypes

import concourse.bass as bass
import concourse.bacc as bacc
import concourse.mybir as mybir
import concourse.tile as tile
from concourse.bass_utils import run_bass_kernel_spmd

F32 = mybir.dt.float32
BF16 = mybir.dt.bfloat16

N_CORES = 8
NB = 2          # batches per core
C = 512
G = 4           # groups
CG = 128        # channels per group
H = 64
W = 64
HW = H * W
MIP = 16        # conv1 output channels
EPS = 1e-5
NSTEP = NB * G  # pipeline iterations per core

# Partition p holds input channel cc = PERM[p] (within its group).
# p = 32*r + q  <->  cc = 4*q + r, so that output channels are contiguous.
PERM = np.array([4 * (p % 32) + p // 32 for p in range(CG)], dtype=np.int64)

_NC_CACHE = None


def _build_bass():
    nc = bacc.Bacc(None, target_bir_lowering=False)

    x_d = nc.dram_tensor("x", [NB, C, H, W], BF16, kind="ExternalInput")
    w1t_d = nc.dram_tensor("w1t", [CG, MIP], BF16, kind="ExternalInput")
    wht_d = nc.dram_tensor("wht", [MIP, CG], BF16, kind="ExternalInput")
    wwt_d = nc.dram_tensor("wwt", [MIP, CG], BF16, kind="ExternalInput")
    bact_d = nc.dram_tensor("bact", [MIP, 1], F32, kind="ExternalInput")
    bh_d = nc.dram_tensor("bh", [CG, 1], F32, kind="ExternalInput")
    bw_d = nc.dram_tensor("bw", [CG, 1], F32, kind="ExternalInput")
    out_d = nc.dram_tensor("out", [NB, C, H, W], BF16, kind="ExternalOutput")

    Relu = mybir.ActivationFunctionType.Relu
    Sigmoid = mybir.ActivationFunctionType.Sigmoid
    AX = mybir.AxisListType.X
    ADD = mybir.AluOpType.add
    MAX = mybir.AluOpType.max
    MIN = mybir.AluOpType.min
    MULT = mybir.AluOpType.mult

    x_f = x_d[:].rearrange("b c h w -> b c (h w)")
    o_f = out_d[:].rearrange("b c h w -> b c (h w)")

    with tile.TileContext(nc) as tc:
        with (
            tc.tile_pool(name="consts", bufs=1) as consts,
            tc.tile_pool(name="xp", bufs=7) as xp,
            tc.tile_pool(name="op", bufs=3) as op,
            tc.tile_pool(name="ahp", bufs=3) as ahp,
            tc.tile_pool(name="ps", bufs=3, space="PSUM") as ps,
            tc.tile_pool(name="ps2", bufs=2, space="PSUM") as ps2,
            tc.tile_pool(name="sm", bufs=12) as sm,
        ):
            w1t = consts.tile([CG, MIP], BF16)
            nc.sync.dma_start(out=w1t, in_=w1t_d[:])
            wht = consts.tile([MIP, CG], BF16)
            nc.sync.dma_start(out=wht, in_=wht_d[:])
            wwt = consts.tile([MIP, CG], BF16)
            nc.sync.dma_start(out=wwt, in_=wwt_d[:])
            bact = consts.tile([MIP, 1], F32)
            nc.sync.dma_start(out=bact, in_=bact_d[:])
            bh = consts.tile([CG, 1], F32)
            nc.sync.dma_start(out=bh, in_=bh_d[:])
            bw = consts.tile([CG, 1], F32)
            nc.sync.dma_start(out=bw, in_=bw_d[:])

            # Pipeline state per in-flight iteration.
            S = [dict() for _ in range(NSTEP)]

            def stg_load(k):
                bi, g = divmod(k, G)
                # 4 DMAs, each with an affine DRAM stride (channels r, r+4,
                # ...) -> partition block [32r, 32r+32): the non-affine
                # 1-DMA nested pattern defeats the 16-engine descriptor
                # spray (measured ~74 GB/s vs ~340 expected).
                X = xp.tile([CG, HW], BF16, name="X")
                for r in range(4):
                    nc.sync.dma_start(
                        out=X[32 * r : 32 * (r + 1)],
                        in_=x_f[bi, CG * g + r : CG * (g + 1) : 4],
                    )
                S[k]["X"] = X

            def stg_pool_mm(k):
                # Yh[m, h, j] accumulates w-octaves; Yw8[m, j, w] accumulates
                # h-octaves (contiguous slab reads AND contiguous PSUM
                # writes — a strided PSUM out AP costs ~1.7 cyc/row on PE).
                X3 = S[k]["X"].rearrange("p (h w) -> p h w", h=H)
                Yh = ps.tile([MIP, H, 8], F32, name="Yh")
                for j in range(8):
                    nc.tensor.matmul(
                        Yh,
                        w1t,
                        X3[:, :, 8 * j : 8 * (j + 1)],
                        start=(j == 0),
                        stop=(j == 7),
                    )
                Yw8 = ps.tile([MIP, 8, W], F32, name="Yw8")
                for j in range(8):
                    nc.tensor.matmul(
                        Yw8,
                        w1t,
                        X3[:, 8 * j : 8 * (j + 1), :],
                        start=(j == 0),
                        stop=(j == 7),
                    )
                S[k]["Yh"], S[k]["Yw8"] = Yh, Yw8

            def stg_hswish(k):
                # Y = [Yh | Yw] (16, 128); then hswish with T = relu(ybn + 3):
                # ybn * relu6(ybn+3) == (T - 3) * min(T, 6)   (/6 in weights)
                Y = sm.tile([MIP, H + W], F32, name="Y")
                nc.vector.tensor_reduce(
                    out=Y[:, 0:H], in_=S[k]["Yh"], axis=AX, op=ADD
                )
                nc.vector.tensor_reduce(
                    out=Y[:, H:],
                    in_=S[k]["Yw8"].rearrange("p j w -> p w j"),
                    axis=AX,
                    op=ADD,
                )
                T = sm.tile([MIP, H + W], F32, name="T")
                nc.vector.tensor_scalar(
                    out=T, in0=Y, scalar1=bact[:], scalar2=0.0, op0=ADD, op1=MAX
                )
                T6 = sm.tile([MIP, H + W], F32, name="T6")
                nc.vector.tensor_scalar_min(T6, T, 6.0)
                HS = sm.tile([MIP, H + W], BF16, name="HS")
                nc.vector.scalar_tensor_tensor(
                    out=HS, in0=T, scalar=-3.0, in1=T6, op0=ADD, op1=MULT
                )
                S[k]["HS"] = HS

            def stg_att_mm(k):
                AHW_ps = ps2.tile([CG, H + W], F32, name="AHW_ps")
                nc.tensor.matmul(
                    AHW_ps[:, 0:H], wht, S[k]["HS"][:, 0:H], start=True, stop=True
                )
                nc.tensor.matmul(
                    AHW_ps[:, H:], wwt, S[k]["HS"][:, H:], start=True, stop=True
                )
                S[k]["AHW_ps"] = AHW_ps

            def stg_sigmoid(k):
                AHW_ps = S[k]["AHW_ps"]
                # a_w first: it unblocks the first big multiply after ~0.3us,
                # overlapping the 3.7us a_h materialization with TT1.
                AW = sm.tile([CG, W], BF16, name="AW")
                nc.scalar.activation(
                    out=AW, in_=AHW_ps[:, H:], func=Sigmoid, bias=bw[:]
                )
                # a_h sigmoid materialized as the broadcast [cg, h, w]; split
                # in halves so the second big multiply can chase it.
                AH = ahp.tile([CG, H, W], BF16, name="AH")
                for half in range(2):
                    h0 = half * (H // 2)
                    nc.scalar.activation(
                        out=AH[:, h0 : h0 + H // 2],
                        in_=AHW_ps[:, h0 : h0 + H // 2]
                        .unsqueeze(2)
                        .broadcast_to([CG, H // 2, W]),
                        func=Sigmoid,
                        bias=bh[:],
                    )
                S[k]["AH"], S[k]["AW"] = AH, AW

            def stg_mult(k):
                # out = x * a_w[., :, w] * a_h[., h, :]; both tensor_tensor
                # ops keep every operand's inner dim packed bf16 -> DVE 2x.
                X3 = S[k]["X"].rearrange("p (h w) -> p h w", h=H)
                OUT = op.tile([CG, HW], BF16, name="OUT")
                OUTr = OUT.rearrange("p (h w) -> p h w", h=H)
                aw_b = S[k]["AW"].unsqueeze(1).broadcast_to([CG, H, W])
                # first multiply: DVE takes rows 0:48, the (otherwise idle)
                # gpsimd engine takes rows 48:64 in parallel.
                HS_SPLIT = 48
                nc.vector.tensor_tensor(
                    out=OUTr[:, 0:HS_SPLIT],
                    in0=X3[:, 0:HS_SPLIT],
                    in1=aw_b[:, 0:HS_SPLIT],
                    op=MULT,
                )
                nc.gpsimd.tensor_tensor(
                    out=OUTr[:, HS_SPLIT:],
                    in0=X3[:, HS_SPLIT:],
                    in1=aw_b[:, HS_SPLIT:],
                    op=MULT,
                )
                AH = S[k]["AH"]
                for half in range(2):
                    h0 = half * (H // 2)
                    nc.vector.tensor_tensor(
                        out=OUTr[:, h0 : h0 + H // 2],
                        in0=OUTr[:, h0 : h0 + H // 2],
                        in1=AH[:, h0 : h0 + H // 2],
                        op=MULT,
                    )
                S[k]["OUT"] = OUT

            def stg_store(k):
                bi, g = divmod(k, G)
                OUT = S[k]["OUT"]
                # channel shuffle = 4 contiguous writes; triggers on the
                # gpsimd (Pool) SWDGE ring to keep HWDGE engines free.
                for r in range(4):
                    c0 = 128 * r + 32 * g
                    nc.gpsimd.dma_start(
                        out=o_f[bi, c0 : c0 + 32],
                        in_=OUT[32 * r : 32 * (r + 1)],
                    )

            # Software pipeline: stage s of iteration k runs in python step
            # k + OFF[s]. hswish leads the Vector queue each step so HS(k)
            # lands early; att-mm(k) (same step, PE) and the sigmoids (+1)
            # then never gate the next step's Vector work — every other
            # cross-engine edge is >= 1 full step old.
            stages = [
                (stg_load, 0, False),
                (stg_hswish, 2, True),
                (stg_pool_mm, 1, False),
                (stg_att_mm, 2, False),
                (stg_sigmoid, 3, False),
                (stg_mult, 4, False),
                (stg_store, 4, False),
            ]
            maxoff = max(off for _, off, _hp in stages)
            for step in range(NSTEP + maxoff):
                for fn, off, hp in stages:
                    k = step - off
                    if 0 <= k < NSTEP:
                        if hp:
                            # hswish gates the next att-mm: pull its priority
                            # forward so the scheduler runs it at the head of
                            # the Vector queue whenever it is ready.
                            with tc.high_priority(offset=60):
                                fn(k)
                        else:
                            fn(k)

    nc.finalize()
    return nc


def _get_nc():
    global _NC_CACHE
    if _NC_CACHE is None:
        _NC_CACHE = _build_bass()
    return _NC_CACHE


def _prep_weights(W1, b1, gamma, beta, mean, var, Wh, bh, Ww, bw):
    W1 = np.asarray(W1, np.float64)
    b1 = np.asarray(b1, np.float64)
    gamma = np.asarray(gamma, np.float64)
    beta = np.asarray(beta, np.float64)
    mean = np.asarray(mean, np.float64)
    var = np.asarray(var, np.float64)
    Wh = np.asarray(Wh, np.float64)
    Ww = np.asarray(Ww, np.float64)
    bh = np.asarray(bh, np.float64)
    bw = np.asarray(bw, np.float64)

    scale = gamma / np.sqrt(var + EPS)                    # (MIP,)
    w1eff = (W1 * scale[:, None]) / float(W)              # (MIP, CG); mean 1/64
    b1eff = scale * (b1 - mean) + beta                    # (MIP,)
    bact = (b1eff + 3.0).astype(np.float32)[:, None]      # (MIP, 1)

    BF = ml_dtypes.bfloat16
    w1t = np.ascontiguousarray(w1eff.T[PERM, :].astype(BF))            # (CG, MIP)
    wht = np.ascontiguousarray((Wh / 6.0)[PERM, :].T.astype(BF))       # (MIP, CG)
    wwt = np.ascontiguousarray((Ww / 6.0)[PERM, :].T.astype(BF))
    bh_p = np.ascontiguousarray(bh[PERM].astype(np.float32)[:, None])
    bw_p = np.ascontiguousarray(bw[PERM].astype(np.float32)[:, None])
    return w1t, wht, wwt, bact, bh_p, bw_p


def run(inputs: dict, trace: bool = False):
    """Run on 8 NeuronCores. Returns (out [16,512,64,64] fp32, results)."""
    x = np.asarray(inputs["x"], dtype=np.float32)
    n = x.shape[0]
    assert x.shape == (n, C, H, W) and n == N_CORES * NB, x.shape
    x_bf = np.ascontiguousarray(x.astype(ml_dtypes.bfloat16))

    w1t, wht, wwt, bact, bh_p, bw_p = _prep_weights(
        inputs["W1"], inputs["b1"], inputs["gamma"], inputs["beta"],
        inputs["mean"], inputs["var"], inputs["Wh"], inputs["bh"],
        inputs["Ww"], inputs["bw"],
    )

    nc = _get_nc()
    core_ids = list(range(N_CORES))
    in_maps = []
    for k in core_ids:
        in_maps.append(
            {
                "x": np.ascontiguousarray(x_bf[NB * k : NB * (k + 1)]),
                "w1t": w1t,
                "wht": wht,
                "wwt": wwt,
                "bact": bact,
                "bh": bh_p,
                "bw": bw_p,
            }
        )

    res = run_bass_kernel_spmd(nc, in_maps, core_ids, trace=trace)
    out = np.concatenate(
        [res.results[k]["out"].astype(np.float32) for k in core_ids], axis=0
    )
    return out, res


def kernel(**inputs) -> np.ndarray:
    out, _ = run(inputs, trace=False)
    return out


def exec_time_ns(res):
    return res.exec_time_ns


# revision 15
# speedup vs baseline: 1.0807x; 1.0599x over previous
"""JointAtt (dense_cnn) Trainium2 Bass kernel — bf16 pipelined version.

Reference computation (per batch n, group g of 4, cg=128 channels, 64x64):
    gh = mean_w x          # (cg, h)
    gw = mean_h x          # (cg, w)
    y  = BN(W1 @ concat(gh, gw) + b1)        # (16, h+w)
    y  = hswish(y) = y * relu6(y+3)/6
    a_h = sigmoid(Wh @ y[:, :h] + bh)        # (cg, h)
    a_w = sigmoid(Ww @ y[:, h:] + bw)        # (cg, w)
    out = x * a_h[:, :, None] * a_w[:, None, :]
    followed by channel shuffle: c' = (c % 4) * 128 + c // 4

Kernel strategy (8 NeuronCores, data-parallel over batch: 2 batches/core):
  - x and out travel as bf16 (host converts): halves HBM traffic and
    enables the DVE 2x_1p mode on the big elementwise multiplies.
  - Per (n, g) slice: x loaded as SBUF [128, 4096] bf16, channel order
    permuted so the final store is the channel shuffle applied contiguously
    (weights permuted on the host to match).
  - Pooling sums fused with the conv1 contraction on the TensorEngine
    (PSUM accumulation, bf16 full rate). Yh accumulates w-octaves with
    n=(h,8w) reads; Yw accumulates h-octaves with fully contiguous
    n=(8h,w) slab reads (h-residues reduced later on DVE).
  - BN scale/bias, the 1/64 pooling mean and the 1/6 hswish divisor are
    folded into the weights on the host.
  - hswish entirely on DVE: T = max(Y+b+3, 0); HS = (T-3)*min(T,6).
  - a_h sigmoid materialized as a broadcast [128, 64, 64] straight out of
    PSUM on the Activation engine (one op); a_w kept [128, 64] and fed to
    DVE as a broadcast access pattern (inner dim stays packed -> 2x mode).
  - Software pipeline with per-stage step offsets so no engine queue
    head-blocks: load(k) | pool-mm(k-1) | reduce+hswish(k-2) |
    att-mm(k-3) | sigmoids(k-4) | big-mults(k-5) | store(k-5).
"""

import numpy as np
import ml_dtypes

import concourse.bass as bass
import concourse.bacc as bacc
import concourse.mybir as mybir
import concourse.tile as tile
from concourse.bass_utils import run_bass_kernel_spmd

F32 = mybir.dt.float32
BF16 = mybir.dt.bfloat16

N_CORES = 8
NB = 2          # batches per core
C = 512
G = 4           # groups
CG = 128        # channels per group
H = 64
W = 64
HW = H * W
MIP = 16        # conv1 output channels
EPS = 1e-5
NSTEP = NB * G  # pipeline iterations per core

# Partition p holds input channel cc = PERM[p] (within its group).
# p = 32*r + q  <->  cc = 4*q + r, so that output channels are contiguous.
PERM = np.array([4 * (p % 32) + p // 32 for p in range(CG)], dtype=np.int64)

_NC_CACHE = None


def _build_bass():
    nc = bacc.Bacc(None, target_bir_lowering=False)

    x_d = nc.dram_tensor("x", [NB, C, H, W], BF16, kind="ExternalInput")
    w1t_d = nc.dram_tensor("w1t", [CG, MIP], BF16, kind="ExternalInput")
    wht_d = nc.dram_tensor("wht", [MIP, CG], BF16, kind="ExternalInput")
    wwt_d = nc.dram_tensor("wwt", [MIP, CG], BF16, kind="ExternalInput")
    bact_d = nc.dram_tensor("bact", [MIP, 1], F32, kind="ExternalInput")
    bh_d = nc.dram_tensor("bh", [CG, 1], F32, kind="ExternalInput")
    bw_d = nc.dram_tensor("bw", [CG, 1], F32, kind="ExternalInput")
    out_d = nc.dram_tensor("out", [NB, C, H, W], BF16, kind="ExternalOutput")

    Relu = mybir.ActivationFunctionType.Relu
    Sigmoid = mybir.ActivationFunctionType.Sigmoid
    AX = mybir.AxisListType.X
    ADD = mybir.AluOpType.add
    MAX = mybir.AluOpType.max
    MIN = mybir.AluOpType.min
    MULT = mybir.AluOpType.mult

    x_f = x_d[:].rearrange("b c h w -> b c (h w)")
    o_f = out_d[:].rearrange("b c h w -> b c (h w)")

    with tile.TileContext(nc) as tc:
        with (
            tc.tile_pool(name="consts", bufs=1) as consts,
            tc.tile_pool(name="xp", bufs=7) as xp,
            tc.tile_pool(name="op", bufs=3) as op,
            tc.tile_pool(name="ahp", bufs=3) as ahp,
            tc.tile_pool(name="ps", bufs=3, space="PSUM") as ps,
            tc.tile_pool(name="ps2", bufs=2, space="PSUM") as ps2,
            tc.tile_pool(name="sm", bufs=12) as sm,
        ):
            w1t = consts.tile([CG, MIP], BF16)
            nc.sync.dma_start(out=w1t, in_=w1t_d[:])
            wht = consts.tile([MIP, CG], BF16)
            nc.sync.dma_start(out=wht, in_=wht_d[:])
            wwt = consts.tile([MIP, CG], BF16)
            nc.sync.dma_start(out=wwt, in_=wwt_d[:])
            bact = consts.tile([MIP, 1], F32)
            nc.sync.dma_start(out=bact, in_=bact_d[:])
            bh = consts.tile([CG, 1], F32)
            nc.sync.dma_start(out=bh, in_=bh_d[:])
            bw = consts.tile([CG, 1], F32)
            nc.sync.dma_start(out=bw, in_=bw_d[:])

            # Pipeline state per in-flight iteration.
            S = [dict() for _ in range(NSTEP)]

            def stg_load(k):
                bi, g = divmod(k, G)
                # 4 DMAs, each with an affine DRAM stride (channels r, r+4,
                # ...) -> partition block [32r, 32r+32): the non-affine
                # 1-DMA nested pattern defeats the 16-engine descriptor
                # spray (measured ~74 GB/s vs ~340 expected).
                X = xp.tile([CG, HW], BF16, name="X")
                for r in range(4):
                    nc.sync.dma_start(
                        out=X[32 * r : 32 * (r + 1)],
                        in_=x_f[bi, CG * g + r : CG * (g + 1) : 4],
                    )
                S[k]["X"] = X

            def stg_pool_mm(k):
                # Yh[m, h, j] accumulates w-octaves; Yw8[m, j, w] accumulates
                # h-octaves (contiguous slab reads AND contiguous PSUM
                # writes — a strided PSUM out AP costs ~1.7 cyc/row on PE).
                X3 = S[k]["X"].rearrange("p (h w) -> p h w", h=H)
                Yh = ps.tile([MIP, H, 8], F32, name="Yh")
                for j in range(8):
                    nc.tensor.matmul(
                        Yh,
                        w1t,
                        X3[:, :, 8 * j : 8 * (j + 1)],
                        start=(j == 0),
                        stop=(j == 7),
                    )
                Yw8 = ps.tile([MIP, 8, W], F32, name="Yw8")
                for j in range(8):
                    nc.tensor.matmul(
                        Yw8,
                        w1t,
                        X3[:, 8 * j : 8 * (j + 1), :],
                        start=(j == 0),
                        stop=(j == 7),
                    )
                S[k]["Yh"], S[k]["Yw8"] = Yh, Yw8

            def stg_hswish(k):
                # Y = [Yh | Yw] (16, 128); then hswish with T = relu(ybn + 3):
                # ybn * relu6(ybn+3) == (T - 3) * min(T, 6)   (/6 in weights)
                Y = sm.tile([MIP, H + W], F32, name="Y")
                nc.vector.tensor_reduce(
                    out=Y[:, 0:H], in_=S[k]["Yh"], axis=AX, op=ADD
                )
                nc.vector.tensor_reduce(
                    out=Y[:, H:],
                    in_=S[k]["Yw8"].rearrange("p j w -> p w j"),
                    axis=AX,
                    op=ADD,
                )
                T = sm.tile([MIP, H + W], F32, name="T")
                nc.vector.tensor_scalar(
                    out=T, in0=Y, scalar1=bact[:], scalar2=0.0, op0=ADD, op1=MAX
                )
                T6 = sm.tile([MIP, H + W], F32, name="T6")
                nc.vector.tensor_scalar_min(T6, T, 6.0)
                HS = sm.tile([MIP, H + W], BF16, name="HS")
                nc.vector.scalar_tensor_tensor(
                    out=HS, in0=T, scalar=-3.0, in1=T6, op0=ADD, op1=MULT
                )
                S[k]["HS"] = HS

            def stg_att_mm(k):
                AHW_ps = ps2.tile([CG, H + W], F32, name="AHW_ps")
                nc.tensor.matmul(
                    AHW_ps[:, 0:H], wht, S[k]["HS"][:, 0:H], start=True, stop=True
                )
                nc.tensor.matmul(
                    AHW_ps[:, H:], wwt, S[k]["HS"][:, H:], start=True, stop=True
                )
                S[k]["AHW_ps"] = AHW_ps

            def stg_sigmoid(k):
                AHW_ps = S[k]["AHW_ps"]
                # a_w first: it unblocks the first big multiply after ~0.3us,
                # overlapping the 3.7us a_h materialization with TT1.
                AW = sm.tile([CG, W], BF16, name="AW")
                nc.scalar.activation(
                    out=AW, in_=AHW_ps[:, H:], func=Sigmoid, bias=bw[:]
                )
                # a_h sigmoid materialized as the broadcast [cg, h, w]; split
                # in halves so the second big multiply can chase it.
                AH = ahp.tile([CG, H, W], BF16, name="AH")
                for half in range(2):
                    h0 = half * (H // 2)
                    nc.scalar.activation(
                        out=AH[:, h0 : h0 + H // 2],
                        in_=AHW_ps[:, h0 : h0 + H // 2]
                        .unsqueeze(2)
                        .broadcast_to([CG, H // 2, W]),
                        func=Sigmoid,
                        bias=bh[:],
                    )
                S[k]["AH"], S[k]["AW"] = AH, AW

            def stg_mult(k):
                # out = x * a_w[., :, w] * a_h[., h, :]; both tensor_tensor
                # ops keep every operand's inner dim packed bf16 -> DVE 2x.
                X3 = S[k]["X"].rearrange("p (h w) -> p h w", h=H)
                OUT = op.tile([CG, HW], BF16, name="OUT")
                OUTr = OUT.rearrange("p (h w) -> p h w", h=H)
                aw_b = S[k]["AW"].unsqueeze(1).broadcast_to([CG, H, W])
                nc.vector.tensor_tensor(out=OUTr, in0=X3, in1=aw_b, op=MULT)
                AH = S[k]["AH"]
                for half in range(2):
                    h0 = half * (H // 2)
                    nc.vector.tensor_tensor(
                        out=OUTr[:, h0 : h0 + H // 2],
                        in0=OUTr[:, h0 : h0 + H // 2],
                        in1=AH[:, h0 : h0 + H // 2],
                        op=MULT,
                    )
                S[k]["OUT"] = OUT

            def stg_store(k):
                bi, g = divmod(k, G)
                OUT = S[k]["OUT"]
                # channel shuffle = 4 contiguous writes; triggers on the
                # gpsimd (Pool) SWDGE ring to keep HWDGE engines free.
                for r in range(4):
                    c0 = 128 * r + 32 * g
                    nc.gpsimd.dma_start(
                        out=o_f[bi, c0 : c0 + 32],
                        in_=OUT[32 * r : 32 * (r + 1)],
                    )

            # Software pipeline: stage s of iteration k runs in python step
            # k + OFF[s]. hswish leads the Vector queue each step so HS(k)
            # lands early; att-mm(k) (same step, PE) and the sigmoids (+1)
            # then never gate the next step's Vector work — every other
            # cross-engine edge is >= 1 full step old.
            stages = [
                (stg_load, 0, False),
                (stg_hswish, 2, True),
                (stg_pool_mm, 1, False),
                (stg_att_mm, 2, False),
                (stg_sigmoid, 3, False),
                (stg_mult, 4, False),
                (stg_store, 4, False),
            ]
            maxoff = max(off for _, off, _hp in stages)
            for step in range(NSTEP + maxoff):
                for fn, off, hp in stages:
                    k = step - off
                    if 0 <= k < NSTEP:
                        if hp:
                            # hswish gates the next att-mm: pull its priority
                            # forward so the scheduler runs it at the head of
                            # the Vector queue whenever it is ready.
                            with tc.high_priority(offset=60):
                                fn(k)
                        else:
                            fn(k)

    nc.finalize()
    return nc


def _get_nc():
    global _NC_CACHE
    if _NC_CACHE is None:
        _NC_CACHE = _build_bass()
    return _NC_CACHE


def _prep_weights(W1, b1, gamma, beta, mean, var, Wh, bh, Ww, bw):
    W1 = np.asarray(W1, np.float64)
    b1 = np.asarray(b1, np.float64)
    gamma = np.asarray(gamma, np.float64)
    beta = np.asarray(beta, np.float64)
    mean = np.asarray(mean, np.float64)
    var = np.asarray(var, np.float64)
    Wh = np.asarray(Wh, np.float64)
    Ww = np.asarray(Ww, np.float64)
    bh = np.asarray(bh, np.float64)
    bw = np.asarray(bw, np.float64)

    scale = gamma / np.sqrt(var + EPS)                    # (MIP,)
    w1eff = (W1 * scale[:, None]) / float(W)              # (MIP, CG); mean 1/64
    b1eff = scale * (b1 - mean) + beta                    # (MIP,)
    bact = (b1eff + 3.0).astype(np.float32)[:, None]      # (MIP, 1)

    BF = ml_dtypes.bfloat16
    w1t = np.ascontiguousarray(w1eff.T[PERM, :].astype(BF))            # (CG, MIP)
    wht = np.ascontiguousarray((Wh / 6.0)[PERM, :].T.astype(BF))       # (MIP, CG)
    wwt = np.ascontiguousarray((Ww / 6.0)[PERM, :].T.astype(BF))
    bh_p = np.ascontiguousarray(bh[PERM].astype(np.float32)[:, None])
    bw_p = np.ascontiguousarray(bw[PERM].astype(np.float32)[:, None])
    return w1t, wht, wwt, bact, bh_p, bw_p


def run(inputs: dict, trace: bool = False):
    """Run on 8 NeuronCores. Returns (out [16,512,64,64] fp32, results)."""
    x = np.asarray(inputs["x"], dtype=np.float32)
    n = x.shape[0]
    assert x.shape == (n, C, H, W) and n == N_CORES * NB, x.shape
    x_bf = np.ascontiguousarray(x.astype(ml_dtypes.bfloat16))

    w1t, wht, wwt, bact, bh_p, bw_p = _prep_weights(
        inputs["W1"], inputs["b1"], inputs["gamma"], inputs["beta"],
        inputs["mean"], inputs["var"], inputs["Wh"], inputs["bh"],
        inputs["Ww"], inputs["bw"],
    )

    nc = _get_nc()
    core_ids = list(range(N_CORES))
    in_maps = []
    for k in core_ids:
        in_maps.append(
            {
                "x": np.ascontiguousarray(x_bf[NB * k : NB * (k + 1)]),
                "w1t": w1t,
                "wht": wht,
                "wwt": wwt,
                "bact": bact,
                "bh": bh_p,
                "bw": bw_p,
            }
        )

    res = run_bass_kernel_spmd(nc, in_maps, core_ids, trace=trace)
    out = np.concatenate(
        [res.results[k]["out"].astype(np.float32) for k in core_ids], axis=0
    )
    return out, res


def kernel(**inputs) -> np.ndarray:
    out, _ = run(inputs, trace=False)
    return out


def exec_time_ns(res):
    return res.exec_time_ns


# revision 18
# speedup vs baseline: 1.1008x; 1.0186x over previous
"""JointAtt (dense_cnn) Trainium2 Bass kernel — bf16 pipelined version.

Reference computation (per batch n, group g of 4, cg=128 channels, 64x64):
    gh = mean_w x          # (cg, h)
    gw = mean_h x          # (cg, w)
    y  = BN(W1 @ concat(gh, gw) + b1)        # (16, h+w)
    y  = hswish(y) = y * relu6(y+3)/6
    a_h = sigmoid(Wh @ y[:, :h] + bh)        # (cg, h)
    a_w = sigmoid(Ww @ y[:, h:] + bw)        # (cg, w)
    out = x * a_h[:, :, None] * a_w[:, None, :]
    followed by channel shuffle: c' = (c % 4) * 128 + c // 4

Kernel strategy (8 NeuronCores, data-parallel over batch: 2 batches/core):
  - x and out travel as bf16 (host converts): halves HBM traffic and
    enables the DVE 2x_1p mode on the big elementwise multiplies.
  - Per (n, g) slice: x loaded as SBUF [128, 4096] bf16, channel order
    permuted so the final store is the channel shuffle applied contiguously
    (weights permuted on the host to match).
  - Pooling sums fused with the conv1 contraction on the TensorEngine
    (PSUM accumulation, bf16 full rate). Yh accumulates w-octaves with
    n=(h,8w) reads; Yw accumulates h-octaves with fully contiguous
    n=(8h,w) slab reads (h-residues reduced later on DVE).
  - BN scale/bias, the 1/64 pooling mean and the 1/6 hswish divisor are
    folded into the weights on the host.
  - hswish entirely on DVE: T = max(Y+b+3, 0); HS = (T-3)*min(T,6).
  - a_h sigmoid materialized as a broadcast [128, 64, 64] straight out of
    PSUM on the Activation engine (one op); a_w kept [128, 64] and fed to
    DVE as a broadcast access pattern (inner dim stays packed -> 2x mode).
  - Software pipeline with per-stage step offsets so no engine queue
    head-blocks: load(k) | pool-mm(k-1) | reduce+hswish(k-2) |
    att-mm(k-3) | sigmoids(k-4) | big-mults(k-5) | store(k-5).
"""

import numpy as np
import ml_dtypes

import concourse.bass as bass
import concourse.bacc as bacc
import concourse.mybir as mybir
import concourse.tile as tile
from concourse.bass_utils import run_bass_kernel_spmd

F32 = mybir.dt.float32
BF16 = mybir.dt.bfloat16

N_CORES = 8
NB = 2          # batches per core
C = 512
G = 4           # groups
CG = 128        # channels per group
H = 64
W = 64
HW = H * W
MIP = 16        # conv1 output channels
EPS = 1e-5
NSTEP = NB * G  # pipeline iterations per core

# Partition p holds input channel cc = PERM[p] (within its group).
# p = 32*r + q  <->  cc = 4*q + r, so that output channels are contiguous.
PERM = np.array([4 * (p % 32) + p // 32 for p in range(CG)], dtype=np.int64)

_NC_CACHE = None


def _build_bass():
    nc = bacc.Bacc(None, target_bir_lowering=False)

    x_d = nc.dram_tensor("x", [NB, C, H, W], BF16, kind="ExternalInput")
    w1t_d = nc.dram_tensor("w1t", [CG, MIP], BF16, kind="ExternalInput")
    wht_d = nc.dram_tensor("wht", [MIP, CG], BF16, kind="ExternalInput")
    wwt_d = nc.dram_tensor("wwt", [MIP, CG], BF16, kind="ExternalInput")
    bact_d = nc.dram_tensor("bact", [MIP, 1], F32, kind="ExternalInput")
    bh_d = nc.dram_tensor("bh", [CG, 1], F32, kind="ExternalInput")
    bw_d = nc.dram_tensor("bw", [CG, 1], F32, kind="ExternalInput")
    out_d = nc.dram_tensor("out", [NB, C, H, W], BF16, kind="ExternalOutput")

    Relu = mybir.ActivationFunctionType.Relu
    Sigmoid = mybir.ActivationFunctionType.Sigmoid
    AX = mybir.AxisListType.X
    ADD = mybir.AluOpType.add
    MAX = mybir.AluOpType.max
    MIN = mybir.AluOpType.min
    MULT = mybir.AluOpType.mult

    x_f = x_d[:].rearrange("b c h w -> b c (h w)")
    o_f = out_d[:].rearrange("b c h w -> b c (h w)")

    with tile.TileContext(nc) as tc:
        with (
            tc.tile_pool(name="consts", bufs=1) as consts,
            tc.tile_pool(name="xp", bufs=8) as xp,
            tc.tile_pool(name="op", bufs=3) as op,
            tc.tile_pool(name="ahp", bufs=3) as ahp,
            tc.tile_pool(name="ps", bufs=3, space="PSUM") as ps,
            tc.tile_pool(name="ps2", bufs=2, space="PSUM") as ps2,
            tc.tile_pool(name="sm", bufs=12) as sm,
        ):
            # consts go on the scalar HWDGE ring so the first X loads on the
            # sync ring start immediately.
            w1t = consts.tile([CG, MIP], BF16)
            nc.scalar.dma_start(out=w1t, in_=w1t_d[:])
            wht = consts.tile([MIP, CG], BF16)
            nc.scalar.dma_start(out=wht, in_=wht_d[:])
            wwt = consts.tile([MIP, CG], BF16)
            nc.scalar.dma_start(out=wwt, in_=wwt_d[:])
            bact = consts.tile([MIP, 1], F32)
            nc.scalar.dma_start(out=bact, in_=bact_d[:])
            bh = consts.tile([CG, 1], F32)
            nc.scalar.dma_start(out=bh, in_=bh_d[:])
            bw = consts.tile([CG, 1], F32)
            nc.scalar.dma_start(out=bw, in_=bw_d[:])

            # Pipeline state per in-flight iteration.
            S = [dict() for _ in range(NSTEP)]

            def stg_load(k):
                bi, g = divmod(k, G)
                # 4 DMAs, each with an affine DRAM stride (channels r, r+4,
                # ...) -> partition block [32r, 32r+32): the non-affine
                # 1-DMA nested pattern defeats the 16-engine descriptor
                # spray (measured ~74 GB/s vs ~340 expected).
                X = xp.tile([CG, HW], BF16, name="X")
                for r in range(4):
                    nc.sync.dma_start(
                        out=X[32 * r : 32 * (r + 1)],
                        in_=x_f[bi, CG * g + r : CG * (g + 1) : 4],
                    )
                S[k]["X"] = X

            def stg_pool_mm(k):
                # Yh[m, h, j] accumulates w-octaves; Yw8[m, j, w] accumulates
                # h-octaves (contiguous slab reads AND contiguous PSUM
                # writes — a strided PSUM out AP costs ~1.7 cyc/row on PE).
                X3 = S[k]["X"].rearrange("p (h w) -> p h w", h=H)
                Yh = ps.tile([MIP, H, 8], F32, name="Yh")
                for j in range(8):
                    nc.tensor.matmul(
                        Yh,
                        w1t,
                        X3[:, :, 8 * j : 8 * (j + 1)],
                        start=(j == 0),
                        stop=(j == 7),
                    )
                Yw8 = ps.tile([MIP, 8, W], F32, name="Yw8")
                for j in range(8):
                    nc.tensor.matmul(
                        Yw8,
                        w1t,
                        X3[:, 8 * j : 8 * (j + 1), :],
                        start=(j == 0),
                        stop=(j == 7),
                    )
                S[k]["Yh"], S[k]["Yw8"] = Yh, Yw8

            def stg_hswish(k):
                # Y = [Yh | Yw] (16, 128); then hswish with T = relu(ybn + 3):
                # ybn * relu6(ybn+3) == (T - 3) * min(T, 6)   (/6 in weights)
                Y = sm.tile([MIP, H + W], F32, name="Y")
                nc.vector.tensor_reduce(
                    out=Y[:, 0:H], in_=S[k]["Yh"], axis=AX, op=ADD
                )
                nc.vector.tensor_reduce(
                    out=Y[:, H:],
                    in_=S[k]["Yw8"].rearrange("p j w -> p w j"),
                    axis=AX,
                    op=ADD,
                )
                T = sm.tile([MIP, H + W], F32, name="T")
                nc.vector.tensor_scalar(
                    out=T, in0=Y, scalar1=bact[:], scalar2=0.0, op0=ADD, op1=MAX
                )
                T6 = sm.tile([MIP, H + W], F32, name="T6")
                nc.vector.tensor_scalar_min(T6, T, 6.0)
                HS = sm.tile([MIP, H + W], BF16, name="HS")
                nc.vector.scalar_tensor_tensor(
                    out=HS, in0=T, scalar=-3.0, in1=T6, op0=ADD, op1=MULT
                )
                S[k]["HS"] = HS

            def stg_att_mm(k):
                AHW_ps = ps2.tile([CG, H + W], F32, name="AHW_ps")
                nc.tensor.matmul(
                    AHW_ps[:, 0:H], wht, S[k]["HS"][:, 0:H], start=True, stop=True
                )
                nc.tensor.matmul(
                    AHW_ps[:, H:], wwt, S[k]["HS"][:, H:], start=True, stop=True
                )
                S[k]["AHW_ps"] = AHW_ps

            def stg_sigmoid(k):
                AHW_ps = S[k]["AHW_ps"]
                # a_w first: it unblocks the first big multiply after ~0.3us,
                # overlapping the 3.7us a_h materialization with TT1.
                AW = sm.tile([CG, W], BF16, name="AW")
                nc.scalar.activation(
                    out=AW, in_=AHW_ps[:, H:], func=Sigmoid, bias=bw[:]
                )
                # a_h sigmoid materialized as the broadcast [cg, h, w]; split
                # in halves so the second big multiply can chase it.
                AH = ahp.tile([CG, H, W], BF16, name="AH")
                for half in range(2):
                    h0 = half * (H // 2)
                    nc.scalar.activation(
                        out=AH[:, h0 : h0 + H // 2],
                        in_=AHW_ps[:, h0 : h0 + H // 2]
                        .unsqueeze(2)
                        .broadcast_to([CG, H // 2, W]),
                        func=Sigmoid,
                        bias=bh[:],
                    )
                S[k]["AH"], S[k]["AW"] = AH, AW

            def stg_mult(k):
                # out = x * a_w[., :, w] * a_h[., h, :]; both tensor_tensor
                # ops keep every operand's inner dim packed bf16 -> DVE 2x.
                X3 = S[k]["X"].rearrange("p (h w) -> p h w", h=H)
                OUT = op.tile([CG, HW], BF16, name="OUT")
                OUTr = OUT.rearrange("p (h w) -> p h w", h=H)
                aw_b = S[k]["AW"].unsqueeze(1).broadcast_to([CG, H, W])
                nc.vector.tensor_tensor(out=OUTr, in0=X3, in1=aw_b, op=MULT)
                AH = S[k]["AH"]
                for half in range(2):
                    h0 = half * (H // 2)
                    nc.vector.tensor_tensor(
                        out=OUTr[:, h0 : h0 + H // 2],
                        in0=OUTr[:, h0 : h0 + H // 2],
                        in1=AH[:, h0 : h0 + H // 2],
                        op=MULT,
                    )
                S[k]["OUT"] = OUT

            def stg_store(k):
                bi, g = divmod(k, G)
                OUT = S[k]["OUT"]
                # channel shuffle = 4 contiguous writes; triggers on the
                # gpsimd (Pool) SWDGE ring to keep HWDGE engines free.
                for r in range(4):
                    c0 = 128 * r + 32 * g
                    nc.gpsimd.dma_start(
                        out=o_f[bi, c0 : c0 + 32],
                        in_=OUT[32 * r : 32 * (r + 1)],
                    )

            # Software pipeline: stage s of iteration k runs in python step
            # k + OFF[s]. hswish leads the Vector queue each step so HS(k)
            # lands early; att-mm(k) (same step, PE) and the sigmoids (+1)
            # then never gate the next step's Vector work — every other
            # cross-engine edge is >= 1 full step old.
            stages = [
                (stg_load, 0, False),
                (stg_hswish, 2, True),
                (stg_pool_mm, 1, False),
                (stg_att_mm, 4, False),
                (stg_sigmoid, 5, False),
                (stg_mult, 6, False),
                (stg_store, 6, False),
            ]
            maxoff = max(off for _, off, _hp in stages)
            for step in range(NSTEP + maxoff):
                for fn, off, hp in stages:
                    k = step - off
                    if 0 <= k < NSTEP:
                        if hp:
                            # hswish gates the next att-mm: pull its priority
                            # forward so the scheduler runs it at the head of
                            # the Vector queue whenever it is ready.
                            with tc.high_priority(offset=60):
                                fn(k)
                        else:
                            fn(k)

    nc.finalize()
    return nc


def _get_nc():
    global _NC_CACHE
    if _NC_CACHE is None:
        _NC_CACHE = _build_bass()
    return _NC_CACHE


def _prep_weights(W1, b1, gamma, beta, mean, var, Wh, bh, Ww, bw):
    W1 = np.asarray(W1, np.float64)
    b1 = np.asarray(b1, np.float64)
    gamma = np.asarray(gamma, np.float64)
    beta = np.asarray(beta, np.float64)
    mean = np.asarray(mean, np.float64)
    var = np.asarray(var, np.float64)
    Wh = np.asarray(Wh, np.float64)
    Ww = np.asarray(Ww, np.float64)
    bh = np.asarray(bh, np.float64)
    bw = np.asarray(bw, np.float64)

    scale = gamma / np.sqrt(var + EPS)                    # (MIP,)
    w1eff = (W1 * scale[:, None]) / float(W)              # (MIP, CG); mean 1/64
    b1eff = scale * (b1 - mean) + beta                    # (MIP,)
    bact = (b1eff + 3.0).astype(np.float32)[:, None]      # (MIP, 1)

    BF = ml_dtypes.bfloat16
    w1t = np.ascontiguousarray(w1eff.T[PERM, :].astype(BF))            # (CG, MIP)
    wht = np.ascontiguousarray((Wh / 6.0)[PERM, :].T.astype(BF))       # (MIP, CG)
    wwt = np.ascontiguousarray((Ww / 6.0)[PERM, :].T.astype(BF))
    bh_p = np.ascontiguousarray(bh[PERM].astype(np.float32)[:, None])
    bw_p = np.ascontiguousarray(bw[PERM].astype(np.float32)[:, None])
    return w1t, wht, wwt, bact, bh_p, bw_p


def run(inputs: dict, trace: bool = False):
    """Run on 8 NeuronCores. Returns (out [16,512,64,64] fp32, results)."""
    x = np.asarray(inputs["x"], dtype=np.float32)
    n = x.shape[0]
    assert x.shape == (n, C, H, W) and n == N_CORES * NB, x.shape
    x_bf = np.ascontiguousarray(x.astype(ml_dtypes.bfloat16))

    w1t, wht, wwt, bact, bh_p, bw_p = _prep_weights(
        inputs["W1"], inputs["b1"], inputs["gamma"], inputs["beta"],
        inputs["mean"], inputs["var"], inputs["Wh"], inputs["bh"],
        inputs["Ww"], inputs["bw"],
    )

    nc = _get_nc()
    core_ids = list(range(N_CORES))
    in_maps = []
    for k in core_ids:
        in_maps.append(
            {
                "x": np.ascontiguousarray(x_bf[NB * k : NB * (k + 1)]),
                "w1t": w1t,
                "wht": wht,
                "wwt": wwt,
                "bact": bact,
                "bh": bh_p,
                "bw": bw_p,
            }
        )

    res = run_bass_kernel_spmd(nc, in_maps, core_ids, trace=trace)
    out = np.concatenate(
        [res.results[k]["out"].astype(np.float32) for k in core_ids], axis=0
    )
    return out, res


def kernel(**inputs) -> np.ndarray:
    out, _ = run(inputs, trace=False)
    return out


def exec_time_ns(res):
    return res.exec_time_ns


# revision 19
# speedup vs baseline: 1.1278x; 1.0246x over previous
"""JointAtt (dense_cnn) Trainium2 Bass kernel — bf16 pipelined version.

Reference computation (per batch n, group g of 4, cg=128 channels, 64x64):
    gh = mean_w x          # (cg, h)
    gw = mean_h x          # (cg, w)
    y  = BN(W1 @ concat(gh, gw) + b1)        # (16, h+w)
    y  = hswish(y) = y * relu6(y+3)/6
    a_h = sigmoid(Wh @ y[:, :h] + bh)        # (cg, h)
    a_w = sigmoid(Ww @ y[:, h:] + bw)        # (cg, w)
    out = x * a_h[:, :, None] * a_w[:, None, :]
    followed by channel shuffle: c' = (c % 4) * 128 + c // 4

Kernel strategy (8 NeuronCores, data-parallel over batch: 2 batches/core):
  - x and out travel as bf16 (host converts): halves HBM traffic and
    enables the DVE 2x_1p mode on the big elementwise multiplies.
  - Per (n, g) slice: x loaded as SBUF [128, 4096] bf16, channel order
    permuted so the final store is the channel shuffle applied contiguously
    (weights permuted on the host to match).
  - Pooling sums fused with the conv1 contraction on the TensorEngine
    (PSUM accumulation, bf16 full rate). Yh accumulates w-octaves with
    n=(h,8w) reads; Yw accumulates h-octaves with fully contiguous
    n=(8h,w) slab reads (h-residues reduced later on DVE).
  - BN scale/bias, the 1/64 pooling mean and the 1/6 hswish divisor are
    folded into the weights on the host.
  - hswish entirely on DVE: T = max(Y+b+3, 0); HS = (T-3)*min(T,6).
  - a_h sigmoid materialized as a broadcast [128, 64, 64] straight out of
    PSUM on the Activation engine (one op); a_w kept [128, 64] and fed to
    DVE as a broadcast access pattern (inner dim stays packed -> 2x mode).
  - Software pipeline with per-stage step offsets so no engine queue
    head-blocks: load(k) | pool-mm(k-1) | reduce+hswish(k-2) |
    att-mm(k-3) | sigmoids(k-4) | big-mults(k-5) | store(k-5).
"""

import numpy as np
import ml_dtypes

import concourse.bass as bass
import concourse.bacc as bacc
import concourse.mybir as mybir
import concourse.tile as tile
from concourse.bass_utils import run_bass_kernel_spmd

F32 = mybir.dt.float32
BF16 = mybir.dt.bfloat16

N_CORES = 8
NB = 2          # batches per core
C = 512
G = 4           # groups
CG = 128        # channels per group
H = 64
W = 64
HW = H * W
MIP = 16        # conv1 output channels
EPS = 1e-5
NSTEP = NB * G  # pipeline iterations per core

# Partition p holds input channel cc = PERM[p] (within its group).
# p = 32*r + q  <->  cc = 4*q + r, so that output channels are contiguous.
PERM = np.array([4 * (p % 32) + p // 32 for p in range(CG)], dtype=np.int64)

_NC_CACHE = None


def _build_bass():
    nc = bacc.Bacc(None, target_bir_lowering=False)

    x_d = nc.dram_tensor("x", [NB, C, H, W], BF16, kind="ExternalInput")
    w1t_d = nc.dram_tensor("w1t", [CG, MIP], BF16, kind="ExternalInput")
    wht_d = nc.dram_tensor("wht", [MIP, CG], BF16, kind="ExternalInput")
    wwt_d = nc.dram_tensor("wwt", [MIP, CG], BF16, kind="ExternalInput")
    bact_d = nc.dram_tensor("bact", [MIP, 1], F32, kind="ExternalInput")
    bh_d = nc.dram_tensor("bh", [CG, 1], F32, kind="ExternalInput")
    bw_d = nc.dram_tensor("bw", [CG, 1], F32, kind="ExternalInput")
    out_d = nc.dram_tensor("out", [NB, C, H, W], BF16, kind="ExternalOutput")

    Relu = mybir.ActivationFunctionType.Relu
    Sigmoid = mybir.ActivationFunctionType.Sigmoid
    AX = mybir.AxisListType.X
    ADD = mybir.AluOpType.add
    MAX = mybir.AluOpType.max
    MIN = mybir.AluOpType.min
    MULT = mybir.AluOpType.mult

    x_f = x_d[:].rearrange("b c h w -> b c (h w)")
    o_f = out_d[:].rearrange("b c h w -> b c (h w)")

    with tile.TileContext(nc) as tc:
        with (
            tc.tile_pool(name="consts", bufs=1) as consts,
            tc.tile_pool(name="xp", bufs=8) as xp,
            tc.tile_pool(name="op", bufs=3) as op,
            tc.tile_pool(name="ahp", bufs=3) as ahp,
            tc.tile_pool(name="ps", bufs=3, space="PSUM") as ps,
            tc.tile_pool(name="ps2", bufs=2, space="PSUM") as ps2,
            tc.tile_pool(name="sm", bufs=12) as sm,
        ):
            # consts go on the scalar HWDGE ring so the first X loads on the
            # sync ring start immediately.
            w1t = consts.tile([CG, MIP], BF16)
            nc.scalar.dma_start(out=w1t, in_=w1t_d[:])
            wht = consts.tile([MIP, CG], BF16)
            nc.scalar.dma_start(out=wht, in_=wht_d[:])
            wwt = consts.tile([MIP, CG], BF16)
            nc.scalar.dma_start(out=wwt, in_=wwt_d[:])
            bact = consts.tile([MIP, 1], F32)
            nc.scalar.dma_start(out=bact, in_=bact_d[:])
            bh = consts.tile([CG, 1], F32)
            nc.scalar.dma_start(out=bh, in_=bh_d[:])
            bw = consts.tile([CG, 1], F32)
            nc.scalar.dma_start(out=bw, in_=bw_d[:])

            # Pipeline state per in-flight iteration.
            S = [dict() for _ in range(NSTEP)]

            def stg_load(k):
                bi, g = divmod(k, G)
                # 4 DMAs, each with an affine DRAM stride (channels r, r+4,
                # ...) -> partition block [32r, 32r+32): the non-affine
                # 1-DMA nested pattern defeats the 16-engine descriptor
                # spray (measured ~74 GB/s vs ~340 expected).
                X = xp.tile([CG, HW], BF16, name="X")
                for r in range(4):
                    nc.sync.dma_start(
                        out=X[32 * r : 32 * (r + 1)],
                        in_=x_f[bi, CG * g + r : CG * (g + 1) : 4],
                    )
                S[k]["X"] = X

            def stg_pool_mm(k):
                # Yh[m, h, j] accumulates w-octaves; Yw8[m, j, w] accumulates
                # h-octaves (contiguous slab reads AND contiguous PSUM
                # writes — a strided PSUM out AP costs ~1.7 cyc/row on PE).
                X3 = S[k]["X"].rearrange("p (h w) -> p h w", h=H)
                Yh = ps.tile([MIP, H, 8], F32, name="Yh")
                for j in range(8):
                    nc.tensor.matmul(
                        Yh,
                        w1t,
                        X3[:, :, 8 * j : 8 * (j + 1)],
                        start=(j == 0),
                        stop=(j == 7),
                    )
                Yw8 = ps.tile([MIP, 8, W], F32, name="Yw8")
                for j in range(8):
                    nc.tensor.matmul(
                        Yw8,
                        w1t,
                        X3[:, 8 * j : 8 * (j + 1), :],
                        start=(j == 0),
                        stop=(j == 7),
                    )
                S[k]["Yh"], S[k]["Yw8"] = Yh, Yw8

            def stg_hswish(k):
                # Y = [Yh | Yw] (16, 128); then hswish with T = relu(ybn + 3):
                # ybn * relu6(ybn+3) == (T - 3) * min(T, 6)   (/6 in weights)
                Y = sm.tile([MIP, H + W], F32, name="Y")
                nc.vector.tensor_reduce(
                    out=Y[:, 0:H], in_=S[k]["Yh"], axis=AX, op=ADD
                )
                nc.vector.tensor_reduce(
                    out=Y[:, H:],
                    in_=S[k]["Yw8"].rearrange("p j w -> p w j"),
                    axis=AX,
                    op=ADD,
                )
                T = sm.tile([MIP, H + W], F32, name="T")
                nc.vector.tensor_scalar(
                    out=T, in0=Y, scalar1=bact[:], scalar2=0.0, op0=ADD, op1=MAX
                )
                T6 = sm.tile([MIP, H + W], F32, name="T6")
                nc.vector.tensor_scalar_min(T6, T, 6.0)
                HS = sm.tile([MIP, H + W], BF16, name="HS")
                nc.vector.scalar_tensor_tensor(
                    out=HS, in0=T, scalar=-3.0, in1=T6, op0=ADD, op1=MULT
                )
                S[k]["HS"] = HS

            def stg_att_mm(k):
                AHW_ps = ps2.tile([CG, H + W], F32, name="AHW_ps")
                nc.tensor.matmul(
                    AHW_ps[:, 0:H], wht, S[k]["HS"][:, 0:H], start=True, stop=True
                )
                nc.tensor.matmul(
                    AHW_ps[:, H:], wwt, S[k]["HS"][:, H:], start=True, stop=True
                )
                S[k]["AHW_ps"] = AHW_ps

            def stg_sigmoid(k):
                AHW_ps = S[k]["AHW_ps"]
                # a_w first: it unblocks the first big multiply after ~0.3us,
                # overlapping the 3.7us a_h materialization with TT1.
                AW = sm.tile([CG, W], BF16, name="AW")
                nc.scalar.activation(
                    out=AW, in_=AHW_ps[:, H:], func=Sigmoid, bias=bw[:]
                )
                # a_h sigmoid materialized as the broadcast [cg, h, w]; split
                # in halves so the second big multiply can chase it.
                AH = ahp.tile([CG, H, W], BF16, name="AH")
                for half in range(2):
                    h0 = half * (H // 2)
                    nc.scalar.activation(
                        out=AH[:, h0 : h0 + H // 2],
                        in_=AHW_ps[:, h0 : h0 + H // 2]
                        .unsqueeze(2)
                        .broadcast_to([CG, H // 2, W]),
                        func=Sigmoid,
                        bias=bh[:],
                    )
                S[k]["AH"], S[k]["AW"] = AH, AW

            def stg_mult(k):
                # out = x * a_w[., :, w] * a_h[., h, :]; both tensor_tensor
                # ops keep every operand's inner dim packed bf16 -> DVE 2x.
                X3 = S[k]["X"].rearrange("p (h w) -> p h w", h=H)
                OUT = op.tile([CG, HW], BF16, name="OUT")
                OUTr = OUT.rearrange("p (h w) -> p h w", h=H)
                aw_b = S[k]["AW"].unsqueeze(1).broadcast_to([CG, H, W])
                nc.vector.tensor_tensor(out=OUTr, in0=X3, in1=aw_b, op=MULT)
                AH = S[k]["AH"]
                for half in range(2):
                    h0 = half * (H // 2)
                    nc.vector.tensor_tensor(
                        out=OUTr[:, h0 : h0 + H // 2],
                        in0=OUTr[:, h0 : h0 + H // 2],
                        in1=AH[:, h0 : h0 + H // 2],
                        op=MULT,
                    )
                S[k]["OUT"] = OUT

            def stg_store(k):
                bi, g = divmod(k, G)
                OUT = S[k]["OUT"]
                # channel shuffle = 4 contiguous writes; triggers on the
                # gpsimd (Pool) SWDGE ring to keep HWDGE engines free.
                for r in range(4):
                    c0 = 128 * r + 32 * g
                    nc.gpsimd.dma_start(
                        out=o_f[bi, c0 : c0 + 32],
                        in_=OUT[32 * r : 32 * (r + 1)],
                    )

            # Software pipeline: stage s of iteration k runs in python step
            # k + OFF[s]. hswish leads the Vector queue each step so HS(k)
            # lands early; att-mm(k) (same step, PE) and the sigmoids (+1)
            # then never gate the next step's Vector work — every other
            # cross-engine edge is >= 1 full step old.
            stages = [
                (stg_load, 0, False),
                (stg_hswish, 2, True),
                (stg_pool_mm, 1, False),
                (stg_att_mm, 4, False),
                (stg_sigmoid, 5, False),
                (stg_mult, 6, False),
                (stg_store, 6, False),
            ]
            # Each python step gets a sim-only minimum timestamp
            # (tile_wait_until) so the Tile scheduler cannot compress the
            # pipeline phasing: its simulated timing diverges from hardware
            # (PE p-states, activation latency), and when it ASAP-packs, the
            # frozen per-engine orders serialize the per-iteration chain.
            STEP_MS = 0.01  # 10us of sim time per pipeline step
            maxoff = max(off for _, off, _hp in stages)
            for step in range(NSTEP + maxoff):
                with tc.tile_wait_until(step * STEP_MS):
                    for fn, off, hp in stages:
                        k = step - off
                        if 0 <= k < NSTEP:
                            if hp:
                                # hswish gates the next att-mm: pull its
                                # priority forward so it leads the Vector
                                # queue whenever it is ready.
                                with tc.high_priority(offset=60):
                                    fn(k)
                            else:
                                fn(k)

    nc.finalize()
    return nc


def _get_nc():
    global _NC_CACHE
    if _NC_CACHE is None:
        _NC_CACHE = _build_bass()
    return _NC_CACHE


def _prep_weights(W1, b1, gamma, beta, mean, var, Wh, bh, Ww, bw):
    W1 = np.asarray(W1, np.float64)
    b1 = np.asarray(b1, np.float64)
    gamma = np.asarray(gamma, np.float64)
    beta = np.asarray(beta, np.float64)
    mean = np.asarray(mean, np.float64)
    var = np.asarray(var, np.float64)
    Wh = np.asarray(Wh, np.float64)
    Ww = np.asarray(Ww, np.float64)
    bh = np.asarray(bh, np.float64)
    bw = np.asarray(bw, np.float64)

    scale = gamma / np.sqrt(var + EPS)                    # (MIP,)
    w1eff = (W1 * scale[:, None]) / float(W)              # (MIP, CG); mean 1/64
    b1eff = scale * (b1 - mean) + beta                    # (MIP,)
    bact = (b1eff + 3.0).astype(np.float32)[:, None]      # (MIP, 1)

    BF = ml_dtypes.bfloat16
    w1t = np.ascontiguousarray(w1eff.T[PERM, :].astype(BF))            # (CG, MIP)
    wht = np.ascontiguousarray((Wh / 6.0)[PERM, :].T.astype(BF))       # (MIP, CG)
    wwt = np.ascontiguousarray((Ww / 6.0)[PERM, :].T.astype(BF))
    bh_p = np.ascontiguousarray(bh[PERM].astype(np.float32)[:, None])
    bw_p = np.ascontiguousarray(bw[PERM].astype(np.float32)[:, None])
    return w1t, wht, wwt, bact, bh_p, bw_p


def run(inputs: dict, trace: bool = False):
    """Run on 8 NeuronCores. Returns (out [16,512,64,64] fp32, results)."""
    x = np.asarray(inputs["x"], dtype=np.float32)
    n = x.shape[0]
    assert x.shape == (n, C, H, W) and n == N_CORES * NB, x.shape
    x_bf = np.ascontiguousarray(x.astype(ml_dtypes.bfloat16))

    w1t, wht, wwt, bact, bh_p, bw_p = _prep_weights(
        inputs["W1"], inputs["b1"], inputs["gamma"], inputs["beta"],
        inputs["mean"], inputs["var"], inputs["Wh"], inputs["bh"],
        inputs["Ww"], inputs["bw"],
    )

    nc = _get_nc()
    core_ids = list(range(N_CORES))
    in_maps = []
    for k in core_ids:
        in_maps.append(
            {
                "x": np.ascontiguousarray(x_bf[NB * k : NB * (k + 1)]),
                "w1t": w1t,
                "wht": wht,
                "wwt": wwt,
                "bact": bact,
                "bh": bh_p,
                "bw": bw_p,
            }
        )

    res = run_bass_kernel_spmd(nc, in_maps, core_ids, trace=trace)
    out = np.concatenate(
        [res.results[k]["out"].astype(np.float32) for k in core_ids], axis=0
    )
    return out, res


def kernel(**inputs) -> np.ndarray:
    out, _ = run(inputs, trace=False)
    return out


def exec_time_ns(res):
    return res.exec_time_ns
